# revision 17
# baseline (speedup 1.0000x reference)
"""Trainium2 Bass kernel for nn_CustomKilLayer (gnn_message_passing).

Math (from the reference):
  - prels is only consumed at row `node_index`, so the relation_pred branch
    needs a single row x = inputs_embeds[token_index[node_index]].
  - M = diag(diagonal(Ac)/deg) makes t = tprev * M diagonal, so
    t @ edges is a per-row scaling of edges by
    tdiag[i] = tprev[i,i] * Ac[i,i] / deg[i].
  - The only large memory traffic is streaming all of A to form the per-row
    sums deg[i] = sum_r w[r] * rowsum(A[r])[i].

This version streams A as fp8 (e4m3) instead of f32, quartering the HBM
traffic (64 MB -> 16 MB per core). The row sums run on the PE array: the A
shard is staged column-major (j on partitions) so a DoubleRow fp8 matmul
with a w-weighted stationary column contracts 256 j-values per instruction,
accumulating deg directly in PSUM. deg sensitivity is tiny (tdiag ~ 1/N and
the output is dominated by the edges[node_index] residual), so fp8
quantization of A and w changes the output by ~1e-6 relative — far inside
the 2e-2 gate. The Ac-diagonal path stays exact f32.

Scheduling notes (engine sequencers are in-order, so any op that waits on
late data blocks every later dispatch on the same queue):
  - A-stream DMAs ride only the sync + gpsimd queues; edges/consts/out use
    the scalar queue so no tail op ever sits in front of a stream dispatch.
  - V = tdiag ⊙ (E @ W_v) + (b_v + res): all transposes/matmuls for every
    row tile run up front, before the stream; each tile's tail is only
    deg -> tdiag -> scale + layernorm on DVE (+ one ACT sqrt).
  - ACT runs only Sqrt in the steady state (table reloads are 1.3 us).
  - The last relation's DMA is split 4-ways so the final DoubleRow matmuls
    chase smaller completion semaphores.

Sharding: rows (node dim) split 512 per core across 8 cores. Each core gets
its fp8-swizzled A shard, f32 diagonal slices of A and tprev, its edges row
shard, and small replicated weights. No collectives; the host concatenates
the 8 output shards.
"""

import os
import sys

import numpy as np

for _p in ("/opt/trn_rl_repo", "/root/.axon_site/_ro/trn_rl_repo"):
    if _p not in sys.path and os.path.isdir(_p):
        sys.path.append(_p)

import concourse.bass as bass
import concourse.bacc as bacc
import concourse.tile as tile
from concourse import mybir
from concourse.masks import make_identity
from concourse import bass_utils

N = 4096          # nodes
D = 256           # embedding dim
R = 8             # relations
NCORES = 8
ROWS = N // NCORES        # 512 rows per core
PT = 128                  # partition tile
TILES = ROWS // PT        # 4 row tiles per core
JG = N // 256             # 16 DoubleRow matmuls cover the 4096-j contraction
KB = D // PT              # 2 contraction blocks of 128 for D=256
LN_EPS = 1e-5
F32 = mybir.dt.float32
F8 = mybir.dt.float8e4
F8NP = mybir.dt.np(F8)    # ml_dtypes.float8_e4m3
ACT = mybir.ActivationFunctionType


def _bcast_mid(ap, n):
    """Insert a stride-0 middle dim of size n into a [P, F] access pattern."""
    return bass.AP(tensor=ap.tensor, offset=ap.offset, ap=[ap.ap[0], [0, n], ap.ap[1]])





def _build_program(repeat=1, astream_bufs=24, dma_engines=("sync", "gpsimd"),
                   last_splits=4):
    nc = bacc.Bacc(
        "TRN2", target_bir_lowering=False, debug=False, num_devices=NCORES
    )

    # fp8 A shard, swizzled so that [it, r] is one [128p, JG, 2, 128i] DMA
    # with j = g*256 + e*128 + p on partitions and the core's row index i on
    # the innermost free dim.
    a8 = nc.dram_tensor("a8_shard", [TILES, R, PT, JG, 2, PT], F8, kind="ExternalInput")
    adiag = nc.dram_tensor("adiag", [ROWS, R], F32, kind="ExternalInput")
    tpd = nc.dram_tensor("tprev_diag", [ROWS], F32, kind="ExternalInput")
    e_sh = nc.dram_tensor("edges_shard", [ROWS, D], F32, kind="ExternalInput")
    wq = nc.dram_tensor("w_q", [D, D], F32, kind="ExternalInput")
    bq = nc.dram_tensor("b_q", [1, D], F32, kind="ExternalInput")
    wv = nc.dram_tensor("w_v", [D, D], F32, kind="ExternalInput")
    bv = nc.dram_tensor("b_v", [1, D], F32, kind="ExternalInput")
    wrel = nc.dram_tensor("wrel", [1, R], F32, kind="ExternalInput")
    rels_in = nc.dram_tensor("rels", [D, R], F32, kind="ExternalInput")
    xrow = nc.dram_tensor("xrow", [1, D], F32, kind="ExternalInput")
    enidx = nc.dram_tensor("edges_nidx", [1, D], F32, kind="ExternalInput")
    out = nc.dram_tensor("out_shard", [ROWS, D], F32, kind="ExternalOutput")

    ts = bass.ts

    with tile.TileContext(nc) as tc:
        with (
            tc.tile_pool(name="pdeg", bufs=2, space="PSUM") as pdeg,
            tc.tile_pool(name="ppre", bufs=1, space="PSUM") as ppre,
            tc.tile_pool(name="ptr", bufs=1, space="PSUM") as ptr,
            tc.tile_pool(name="pet", bufs=2, space="PSUM") as pet,
            tc.tile_pool(name="pvpre", bufs=2, space="PSUM") as pvpre,
            tc.tile_pool(name="consts", bufs=1) as consts,
            tc.tile_pool(name="astream", bufs=astream_bufs) as astream,
            tc.tile_pool(name="small", bufs=1) as small,
            tc.tile_pool(name="pertile", bufs=2) as pertile,
            tc.tile_pool(name="vkeep", bufs=4) as vkeep,
        ):
            # ---- constants / replicated weights (scalar HWDGE queue) ----
            ident = consts.tile([PT, PT], F32)
            make_identity(nc, ident[:])
            ones_row = consts.tile([1, PT], F32)
            nc.vector.memset(ones_row[:], 1.0)
            one1 = consts.tile([1, 1], F32)
            nc.vector.memset(one1[:], 1.0)
            eps1 = consts.tile([1, 1], F32)
            nc.vector.memset(eps1[:], LN_EPS)
            eps128 = consts.tile([PT, 1], F32)
            nc.vector.memset(eps128[:], LN_EPS)

            # warm the activation tables while DMA is idle at t=0; Sqrt last
            # so it is resident for the steady state (the only ACT func used
            # after the prelude)
            junkw = small.tile([1, 4], F32)
            nc.scalar.activation(out=junkw[:, 0:1], in_=eps1[:], func=ACT.Exp)
            nc.scalar.activation(out=junkw[:, 1:2], in_=eps1[:], func=ACT.Sqrt)

            xrowt_sb = consts.tile([PT, KB], F32)
            nc.scalar.dma_start(
                out=xrowt_sb[:], in_=xrow[0, :].rearrange("(a k) -> k a", a=KB)
            )
            wq_sb = consts.tile([PT, KB, D], F32)
            nc.scalar.dma_start(
                out=wq_sb[:], in_=wq[:, :].rearrange("(a k) n -> k a n", a=KB)
            )
            bq_sb = consts.tile([1, D], F32)
            nc.scalar.dma_start(out=bq_sb[:], in_=bq[:, :])
            rels_dp = consts.tile([PT, KB, R], F32)
            nc.scalar.dma_start(
                out=rels_dp[:], in_=rels_in[:, :].rearrange("(a k) r -> k a r", a=KB)
            )
            wrel_sb = consts.tile([1, R], F32)
            nc.scalar.dma_start(out=wrel_sb[:], in_=wrel[:, :])
            wv_sb = consts.tile([PT, KB, D], F32)
            nc.scalar.dma_start(
                out=wv_sb[:], in_=wv[:, :].rearrange("(a k) n -> k a n", a=KB)
            )
            e_all = consts.tile([PT, TILES, D], F32)
            nc.scalar.dma_start(
                out=e_all[:], in_=e_sh[:, :].rearrange("(t p) d -> p t d", p=PT)
            )
            bv_sb = consts.tile([1, D], F32)
            nc.scalar.dma_start(out=bv_sb[:], in_=bv[:, :])
            en_sb = consts.tile([1, D], F32)
            nc.scalar.dma_start(out=en_sb[:], in_=enidx[:, :])
            diag_all = consts.tile([PT, TILES, R], F32)
            nc.scalar.dma_start(
                out=diag_all[:], in_=adiag[:, :].rearrange("(t p) r -> p t r", p=PT)
            )
            tp_all = consts.tile([PT, TILES], F32)
            nc.scalar.dma_start(
                out=tp_all[:], in_=tpd[:].rearrange("(t p) -> p t", p=PT)
            )

            for _rep in range(repeat):
                # ---- relation_pred on one row (tiny; hidden under the A
                # stream prefetch). ppre's single psum bank hosts the four
                # sequential accumulation groups: q, qnT, logits, wb.
                pre_ps = ppre.tile([PT, 512], F32)
                q_ps = pre_ps[0:1, 0:D]
                for a in range(KB):
                    nc.tensor.matmul(
                        q_ps,
                        xrowt_sb[:, a : a + 1],
                        wq_sb[:, a, :],
                        start=(a == 0),
                        stop=(a == KB - 1),
                    )
                qb = small.tile([1, D], F32)
                nc.vector.tensor_add(qb[:], q_ps, bq_sb[:])
                # layernorm of the single q row via bn stats
                qst = small.tile([1, 6], F32)
                nc.vector.bn_stats(out=qst[:], in_=qb[:])
                qmv = small.tile([1, 2], F32)
                nc.vector.bn_aggr(out=qmv[:], in_=qst[:])
                qsd = small.tile([1, 1], F32)
                nc.scalar.activation(
                    out=qsd[:], in_=qmv[:, 1:2], func=ACT.Sqrt, bias=eps1[:]
                )
                qrstd = small.tile([1, 1], F32)
                nc.vector.reciprocal(qrstd[:], qsd[:])
                qn = small.tile([1, D], F32)
                nc.vector.tensor_scalar(
                    out=qn[:],
                    in0=qb[:],
                    scalar1=qmv[:, 0:1],
                    scalar2=qrstd[:],
                    op0=mybir.AluOpType.subtract,
                    op1=mybir.AluOpType.mult,
                )
                # transpose qn to [128, KB] via two K=1 matmuls, then
                # logits[1, R] = qn^T.T @ rels (d contracted on partitions)
                qnT_ps = pre_ps[:, 264:266]
                for a in range(KB):
                    nc.tensor.matmul(
                        qnT_ps[:, a : a + 1],
                        qn[:, ts(a, PT)],
                        one1[:],
                        start=True,
                        stop=True,
                    )
                qnT_sb = small.tile([PT, KB], F32)
                nc.vector.tensor_copy(qnT_sb[:], qnT_ps)
                logits_ps = pre_ps[0:1, 272 : 272 + R]
                for a in range(KB):
                    nc.tensor.matmul(
                        logits_ps,
                        qnT_sb[:, a : a + 1],
                        rels_dp[:, a, :],
                        start=(a == 0),
                        stop=(a == KB - 1),
                    )
                # softmax over R, then w = wrel * prels
                logits = small.tile([1, R], F32)
                nc.vector.tensor_copy(logits[:], logits_ps)
                mx = small.tile([1, 1], F32)
                nc.vector.reduce_max(out=mx[:], in_=logits[:], axis=mybir.AxisListType.X)
                negmx = small.tile([1, 1], F32)
                nc.vector.tensor_scalar_mul(negmx[:], mx[:], -1.0)
                exps = small.tile([1, R], F32)
                sumexp = small.tile([1, 1], F32)
                nc.scalar.activation(
                    out=exps[:],
                    in_=logits[:],
                    func=ACT.Exp,
                    bias=negmx[:],
                    accum_out=sumexp[:],
                )
                rsum = small.tile([1, 1], F32)
                nc.vector.reciprocal(rsum[:], sumexp[:])
                w_sb = small.tile([1, R], F32)
                nc.vector.tensor_scalar(
                    out=w_sb[:],
                    in0=exps[:],
                    scalar1=rsum[:],
                    scalar2=None,
                    op0=mybir.AluOpType.mult,
                )
                nc.vector.tensor_tensor(
                    out=w_sb[:], in0=w_sb[:], in1=wrel_sb[:], op=mybir.AluOpType.mult
                )
                # broadcast w to all 128 partitions via ones[1,128].T @ w[1,R]
                wb_ps = pre_ps[:, 280 : 280 + R]
                nc.tensor.matmul(wb_ps, ones_row[:], w_sb[:], start=True, stop=True)
                wb_sb = small.tile([PT, R], F32)
                nc.vector.tensor_copy(wb_sb[:], wb_ps)
                # fp8 copy of the broadcast weights for the DoubleRow lhsT.
                # The dual-fp8 Ldweights ISA check wants [K, 2, M] with the
                # pair-dim stride a multiple of 16 and M even, so duplicate
                # each w[r] into a column pair: wb8[p, e, r, m] = w[r].
                wb8 = small.tile([PT, 2, R, 2], F8)
                for e in range(2):
                    for m in range(2):
                        nc.vector.tensor_copy(wb8[:, e, :, m], wb_ps)
                # Ac-diagonal path for all tiles at once: acd = (diag ⊙ w).sum(r)
                prodD = small.tile([PT, TILES, R], F32)
                nc.vector.tensor_tensor(
                    out=prodD[:],
                    in0=diag_all[:],
                    in1=_bcast_mid(wb_sb[:], TILES),
                    op=mybir.AluOpType.mult,
                )
                acd_all = small.tile([PT, TILES], F32)
                nc.vector.reduce_sum(
                    out=acd_all[:], in_=prodD[:], axis=mybir.AxisListType.X
                )

                # ---- bias+residual row and all V pre-products up front ----
                bvres_sb = small.tile([1, D], F32)
                nc.vector.tensor_add(bvres_sb[:], bv_sb[:], en_sb[:])
                bvres_ps = pvpre.tile([PT, D], F32, tag="vpre")
                nc.tensor.matmul(
                    bvres_ps[:], ones_row[:], bvres_sb[:], start=True, stop=True
                )
                bvres128 = small.tile([PT, D], F32)
                nc.vector.tensor_copy(bvres128[:], bvres_ps[:])

                vpre_sbs = []
                for it in range(TILES):
                    et_sb = pertile.tile([PT, KB, PT], F32)
                    for j in range(KB):
                        et_ps = pet.tile([PT, PT], F32, tag="et_ps")
                        nc.tensor.transpose(
                            et_ps[:], e_all[:, it, ts(j, PT)], ident[:]
                        )
                        nc.vector.tensor_copy(et_sb[:, j, :], et_ps[:])
                    vpre_ps = pvpre.tile([PT, D], F32, tag="vpre")
                    for j in range(KB):
                        nc.tensor.matmul(
                            vpre_ps[:],
                            et_sb[:, j, :],
                            wv_sb[:, j, :],
                            start=(j == 0),
                            stop=(j == KB - 1),
                        )
                    vpre_sb = vkeep.tile([PT, D], F32)
                    nc.vector.tensor_copy(vpre_sb[:], vpre_ps[:])
                    vpre_sbs.append(vpre_sb)

                # ---- main loop: stream fp8 A, deg via DoubleRow matmuls ----
                for it in range(TILES):
                    deg_ps = pdeg.tile([2, 512], F32)  # full psum bank
                    for r in range(R):
                        last = it == TILES - 1 and r == R - 1
                        splits = last_splits if last else 1
                        gs = JG // splits
                        for s in range(splits):
                            a_t = astream.tile([PT, gs, 2, PT], F8, tag="a_t%d" % splits)
                            eng = getattr(
                                nc, dma_engines[(it * R + r + s) % len(dma_engines)]
                            )
                            eng.dma_start(
                                out=a_t[:],
                                in_=a8[it, r, :, s * gs : (s + 1) * gs],
                            )
                            for g in range(gs):
                                nc.tensor.matmul(
                                    deg_ps[0:2, 0:PT],
                                    wb8[:, :, r, :],
                                    a_t[:, g, :, :],
                                    start=(r == 0 and s == 0 and g == 0),
                                    stop=(last and s == splits - 1 and g == gs - 1),
                                    perf_mode=mybir.MatmulPerfMode.DoubleRow,
                                )

                    # ---- tail for this tile: deg -> tdiag -> scale + LN ----
                    degsb = pertile.tile([1, PT], F32)
                    nc.vector.tensor_copy(degsb[:], deg_ps[0:1, 0:PT])
                    degT_ps = ptr.tile([PT, 1], F32, tag="degT")
                    nc.tensor.matmul(degT_ps[:], degsb[:], one1[:], start=True, stop=True)
                    rdeg_t = pertile.tile([PT, 1], F32)
                    nc.vector.reciprocal(rdeg_t[:], degT_ps[:])
                    tdiag_t = pertile.tile([PT, 1], F32)
                    nc.vector.tensor_scalar(
                        out=tdiag_t[:],
                        in0=acd_all[:, it : it + 1],
                        scalar1=rdeg_t[:],
                        scalar2=tp_all[:, it : it + 1],
                        op0=mybir.AluOpType.mult,
                        op1=mybir.AluOpType.mult,
                    )
                    vlm = pertile.tile([PT, D], F32)
                    nc.vector.tensor_scalar(
                        out=vlm[:],
                        in0=vpre_sbs[it][:],
                        scalar1=tdiag_t[:],
                        scalar2=None,
                        op0=mybir.AluOpType.mult,
                    )
                    nc.vector.tensor_tensor(
                        out=vlm[:], in0=vlm[:], in1=bvres128[:], op=mybir.AluOpType.add
                    )
                    # layernorm rows of vlm
                    stats = pertile.tile([PT, 6], F32)
                    nc.vector.bn_stats(out=stats[:], in_=vlm[:])
                    mv = pertile.tile([PT, 2], F32)
                    nc.vector.bn_aggr(out=mv[:], in_=stats[:])
                    sd_t = pertile.tile([PT, 1], F32)
                    nc.scalar.activation(
                        out=sd_t[:],
                        in_=mv[:, 1:2],
                        func=ACT.Sqrt,
                        bias=eps128[:],
                    )
                    rstd_t = pertile.tile([PT, 1], F32)
                    nc.vector.reciprocal(rstd_t[:], sd_t[:])
                    out_t = pertile.tile([PT, D], F32)
                    nc.vector.tensor_scalar(
                        out=out_t[:],
                        in0=vlm[:],
                        scalar1=mv[:, 0:1],
                        scalar2=rstd_t[:],
                        op0=mybir.AluOpType.subtract,
                        op1=mybir.AluOpType.mult,
                    )
                    nc.scalar.dma_start(out=out[ts(it, PT), :], in_=out_t[:])

    nc.compile()
    return nc


_NC_CACHE = None


def _get_nc():
    global _NC_CACHE
    if _NC_CACHE is None:
        _NC_CACHE = _build_program()
    return _NC_CACHE


def _make_in_maps(inputs):
    f32 = lambda x: np.ascontiguousarray(np.asarray(x), dtype=np.float32)
    inputs_embeds = f32(inputs["inputs_embeds"])
    token_index = np.asarray(inputs["token_index"])
    node_index = int(np.asarray(inputs["node_index"]))
    edges = f32(inputs["edges"])
    A = np.asarray(inputs["A"], dtype=np.float32)
    rels = f32(inputs["rels"])
    wrel = f32(inputs["wrel"]).reshape(1, R)
    W_q = f32(inputs["W_q"])
    b_q = f32(inputs["b_q"]).reshape(1, D)
    W_v = f32(inputs["W_v"])
    b_v = f32(inputs["b_v"]).reshape(1, D)
    tprev = np.asarray(inputs["tprev"], dtype=np.float32)

    row = int(token_index[node_index])
    xrow = np.ascontiguousarray(inputs_embeds[row]).reshape(1, D)
    enidx = np.ascontiguousarray(edges[node_index]).reshape(1, D)
    tprev_diag = np.ascontiguousarray(np.diagonal(tprev))  # [N]
    a_diag = np.ascontiguousarray(
        np.transpose(np.diagonal(A, axis1=1, axis2=2))
    )  # [N, R]

    # fp8 cast once, then per-core swizzle to the PE streaming layout:
    # a8_shard[it, r, p, g, e, i] = A8[r, lo + it*128 + i, g*256 + e*128 + p]
    A8u = A.astype(F8NP).view(np.uint8)           # [R, N, N] bytes

    in_maps = []
    for c in range(NCORES):
        lo, hi = c * ROWS, (c + 1) * ROWS
        a8c = (
            A8u[:, lo:hi, :]
            .reshape(R, TILES, PT, JG, 2, PT)
            .transpose(1, 0, 5, 3, 4, 2)
        )
        a8c = np.ascontiguousarray(a8c).view(F8NP)
        in_maps.append(
            {
                "a8_shard": a8c,
                "adiag": np.ascontiguousarray(a_diag[lo:hi]),
                "tprev_diag": np.ascontiguousarray(tprev_diag[lo:hi]),
                "edges_shard": np.ascontiguousarray(edges[lo:hi]),
                "w_q": W_q,
                "b_q": b_q,
                "w_v": W_v,
                "b_v": b_v,
                "wrel": wrel,
                "rels": rels,
                "xrow": xrow,
                "edges_nidx": enidx,
            }
        )
    return in_maps


def run(trace=False, **inputs):
    """Run the kernel; returns (full_output, BassKernelResults)."""
    nc = _get_nc()
    in_maps = _make_in_maps(inputs)
    res = bass_utils.run_bass_kernel_spmd(
        nc, in_maps, core_ids=list(range(NCORES)), trace=trace
    )
    outp = np.concatenate(
        [np.asarray(res.results[c]["out_shard"]) for c in range(NCORES)], axis=0
    )
    return outp.astype(np.float32), res


def kernel(**inputs):
    outp, _ = run(trace=False, **inputs)
    return outp
